# revision 2
# baseline (speedup 1.0000x reference)
"""Trainium2 Bass kernel for the RetinaFace-style detection loss (nn_LossOD_K).

Strategy: pure data parallel over the batch dim (64 rows -> 8 cores x 8 rows).
Inside each core the 8 rows x 16800 anchors are flattened to 134400 anchors and
laid out as 128 partitions x 1050 anchors (partition p holds row p//16, block
p%16).  All elementwise math is row-independent; per-row sums come from ACT
accum_out (per-partition scalars) plus one block-diagonal PE matmul that
collapses the 16 partitions of each row.

smooth_l1(x) = (|x| - 0.5) + 0.5*relu(1-|x|)^2.  With the {0,1} mask folded
into r (r_masked = r*mask) the per-element value a+q == smooth+0.5 for kept
elements and exactly 0.5 for masked ones, so
    sum(a+q) = sum_masked(smooth) + 0.5*K*A
and the constant 0.5*K*A is subtracted on the host.

OHEM double-argsort: with C=2 and labels ~ Bernoulli(1/2), neg_num =
min(3*pos_num, A) == A for every row, which makes mask_neg all-ones, so
loss_labels = sum(loss) + sum(loss*mask_pos).  This is verified on the host
from the exact pos_num computed on device; in the (practically impossible)
case it fails, an exact numpy fallback recomputes the label loss.
"""

import sys

sys.path.insert(0, "/opt/trn_rl_repo")

from contextlib import ExitStack

import ml_dtypes
import numpy as np

B, A, C, K = 64, 16800, 2, 10
NCORES = 8
RPC = B // NCORES          # rows per core
P = 128                    # partitions
BLK = P // RPC             # 16 partition-blocks per row
LA = A // BLK              # 1050 local anchors per partition
NCH = 7                    # chunks along the free dim
FC = LA // NCH             # 150 anchors per partition per chunk
FLOAT_EPS = float(np.finfo(np.float32).eps)

_CACHE = {}


def _build():
    import concourse.bass as bass
    import concourse.tile as tile
    from concourse import bacc, mybir

    AF = mybir.ActivationFunctionType
    ALU = mybir.AluOpType
    f32, bf16, i32 = mybir.dt.float32, mybir.dt.bfloat16, mybir.dt.int32

    nc = bacc.Bacc("TRN2", target_bir_lowering=False, debug=False,
                   num_devices=NCORES)

    pb = nc.dram_tensor("pb", [RPC, A, 4], f32, kind="ExternalInput")
    gb = nc.dram_tensor("gb", [RPC, A, 4], f32, kind="ExternalInput")
    pl = nc.dram_tensor("pl", [RPC, A, 2], bf16, kind="ExternalInput")
    gl = nc.dram_tensor("gl", [RPC, A], i32, kind="ExternalInput")
    pk = nc.dram_tensor("pk", [RPC, A, K], bf16, kind="ExternalInput")
    gk = nc.dram_tensor("gk", [RPC, A, K], bf16, kind="ExternalInput")
    # host-precomputed per-anchor constants in block layout [BLK, 2*LA]
    cx = nc.dram_tensor("cx", [BLK, 2 * LA], bf16, kind="ExternalInput")
    wk = nc.dram_tensor("wk", [BLK, 2 * LA], bf16, kind="ExternalInput")
    l5c = nc.dram_tensor("l5c", [BLK, 2 * LA], f32, kind="ExternalInput")
    blk8 = nc.dram_tensor("blk8", [P, RPC], f32, kind="ExternalInput")
    out = nc.dram_tensor("out", [RPC, 32], f32, kind="ExternalOutput")

    def rep(dram, width, comps, chunk=None, dtype_sz=1):
        """AP replicating the [BLK, width] block-const across the 8 row groups."""
        off = 0 if chunk is None else chunk * FC * comps
        w = width if chunk is None else FC * comps
        return bass.AP(tensor=dram, offset=off,
                       ap=[[0, RPC], [dram.shape[1], BLK], [1, w]])

    def load(dram, comps, chunk, dt):
        """AP for chunk `chunk` of a [RPC, A, comps] input in flat layout."""
        return bass.AP(tensor=dram, offset=chunk * FC * comps,
                       ap=[[A * comps, RPC], [LA * comps, BLK], [1, FC * comps]])

    with tile.TileContext(nc) as tc, ExitStack() as ctx:
        singles = ctx.enter_context(tc.tile_pool(name="singles", bufs=1))
        ins = ctx.enter_context(tc.tile_pool(name="ins", bufs=3))
        mids = ctx.enter_context(tc.tile_pool(name="mids", bufs=2))
        psum = ctx.enter_context(tc.tile_pool(name="psum", bufs=1, space="PSUM"))

        # ---- constants (loaded once) ----
        cx_t = singles.tile([P, 2 * LA], bf16)
        nc.sync.dma_start(cx_t[:], rep(cx, 2 * LA, 2))
        wk_t = singles.tile([P, 2 * LA], bf16)
        nc.sync.dma_start(wk_t[:], rep(wk, 2 * LA, 2))
        l5c_t = singles.tile([P, 2 * LA], f32)
        nc.sync.dma_start(l5c_t[:], rep(l5c, 2 * LA, 2))
        blk8_t = singles.tile([P, RPC], f32)
        nc.sync.dma_start(blk8_t[:], blk8[:])
        # c2 = 2*cx, w2 = wk/2 (bbox-xy variants of the anchor constants)
        c2_t = singles.tile([P, 2 * LA], bf16)
        nc.vector.tensor_scalar(c2_t[:], cx_t[:], 2.0, None, op0=ALU.mult)
        w2_t = singles.tile([P, 2 * LA], bf16)
        nc.vector.tensor_scalar(w2_t[:], wk_t[:], 0.5, None, op0=ALU.mult)

        # ACT-accumulated per-chunk sums (columns = chunks)
        acc_ab = singles.tile([P, NCH], f32)
        acc_qb = singles.tile([P, NCH], f32)
        acc_ak = singles.tile([P, NCH], f32)
        acc_qk = singles.tile([P, NCH], f32)

        ps_loss = psum.tile([RPC, FC], f32)
        ps_pos = psum.tile([RPC, FC], f32)
        ps_g = psum.tile([RPC, FC], f32)

        for j in range(NCH):
            first, last = j == 0, j == NCH - 1
            # ---- loads ----
            pb_t = ins.tile([P, 4 * FC], f32)
            nc.sync.dma_start(pb_t[:], load(pb, 4, j, f32))
            gb_t = ins.tile([P, 4 * FC], f32)
            nc.sync.dma_start(gb_t[:], load(gb, 4, j, f32))
            pl_t = ins.tile([P, 2 * FC], bf16)
            nc.sync.dma_start(pl_t[:], load(pl, 2, j, bf16))
            gl_t = ins.tile([P, FC], i32)
            nc.sync.dma_start(gl_t[:], load(gl, 1, j, i32))
            pk_t = ins.tile([P, K * FC], bf16)
            nc.sync.dma_start(pk_t[:], load(pk, K, j, bf16))
            gk_t = ins.tile([P, K * FC], bf16)
            nc.sync.dma_start(gk_t[:], load(gk, K, j, bf16))

            def ap(t, off, step, n, inner=None):
                d = [t.ap[0], [step, n]]
                if inner is not None:
                    d.append(inner)
                return bass.AP(tensor=t.tensor, offset=t.offset + off, ap=d)

            # ---- labels ----
            glf = mids.tile([P, FC], f32)
            nc.vector.tensor_copy(glf[:], gl_t[:])
            glfb = mids.tile([P, FC], bf16)
            nc.vector.tensor_copy(glfb[:], glf[:])
            sign = mids.tile([P, FC], bf16)
            nc.vector.tensor_scalar(sign[:], glfb[:], -2.0, 1.0,
                                    op0=ALU.mult, op1=ALU.add)
            diff = mids.tile([P, FC], bf16)
            nc.vector.tensor_tensor(diff[:], ap(pl_t, 1, 2, FC),
                                    ap(pl_t, 0, 2, FC), op=ALU.subtract)
            m = mids.tile([P, FC], bf16)
            nc.vector.tensor_tensor(m[:], diff[:], sign[:], op=ALU.mult)
            relu_m = mids.tile([P, FC], f32)
            nc.scalar.activation(relu_m[:], m[:], AF.Relu)
            am = mids.tile([P, FC], bf16)
            nc.scalar.activation(am[:], m[:], AF.Abs)
            em = mids.tile([P, FC], f32)
            nc.scalar.activation(em[:], am[:], AF.Exp, scale=-1.0)
            lp = mids.tile([P, FC], f32)
            nc.scalar.activation(lp[:], em[:], AF.Ln, bias=1.0)
            loss = mids.tile([P, FC], f32)
            nc.vector.tensor_tensor(loss[:], relu_m[:], lp[:], op=ALU.add)
            pos = mids.tile([P, FC], f32)
            nc.vector.tensor_tensor(pos[:], loss[:], glf[:], op=ALU.mult)
            nc.tensor.matmul(ps_loss[:], blk8_t[:], loss[:], start=first, stop=last)
            nc.tensor.matmul(ps_pos[:], blk8_t[:], pos[:], start=first, stop=last)
            nc.tensor.matmul(ps_g[:], blk8_t[:], glf[:], start=first, stop=last)

            # ---- bbox ----
            s = mids.tile([P, 2 * FC], bf16)     # l+r / t+b per anchor
            nc.vector.tensor_tensor(s[:], ap(gb_t, 0, 4, FC, [1, 2]),
                                    ap(gb_t, 2, 4, FC, [1, 2]), op=ALU.add)
            t1 = mids.tile([P, 2 * FC], bf16)
            nc.vector.tensor_tensor(t1[:], s[:],
                                    c2_t[:, j * 2 * FC:(j + 1) * 2 * FC],
                                    op=ALU.subtract)
            dxy = mids.tile([P, 2 * FC], bf16)
            nc.vector.tensor_tensor(dxy[:], t1[:],
                                    w2_t[:, j * 2 * FC:(j + 1) * 2 * FC],
                                    op=ALU.mult)
            rb = mids.tile([P, 4 * FC], bf16)    # [rxy | rwh]
            nc.vector.tensor_tensor(
                rb[:].rearrange("p (f c) -> p f c", c=4)[:, :, 0:2],
                ap(pb_t, 0, 4, FC, [1, 2]),
                dxy[:].rearrange("p (f c) -> p f c", c=2), op=ALU.subtract)
            gwh = mids.tile([P, 2 * FC], f32)
            nc.vector.tensor_tensor(gwh[:], ap(gb_t, 2, 4, FC, [1, 2]),
                                    ap(gb_t, 0, 4, FC, [1, 2]), op=ALU.subtract)
            lg = mids.tile([P, 2 * FC], f32)
            nc.scalar.activation(lg[:], gwh[:], AF.Ln)
            u = mids.tile([P, 2 * FC], f32)      # p_wh + 5*ln(anc_wh)
            nc.vector.tensor_tensor(u[:], ap(pb_t, 2, 4, FC, [1, 2]),
                                    l5c_t[:, j * 2 * FC:(j + 1) * 2 * FC],
                                    op=ALU.add)
            lg5 = mids.tile([P, 2 * FC], f32)
            nc.vector.tensor_scalar(lg5[:], lg[:], -5.0, None, op0=ALU.mult)
            nc.vector.tensor_tensor(
                rb[:].rearrange("p (f c) -> p f c", c=4)[:, :, 2:4],
                u[:].rearrange("p (f c) -> p f c", c=2),
                lg5[:].rearrange("p (f c) -> p f c", c=2), op=ALU.add)
            # mask: rmb = rb * glf  (broadcast over the 4 comps)
            rmb = mids.tile([P, 4 * FC], bf16)
            glfb_b4 = bass.AP(tensor=glfb.tensor, offset=glfb.offset,
                              ap=[glfb.ap[0], [1, FC], [0, 4]])
            nc.vector.tensor_tensor(
                rmb[:].rearrange("p (f c) -> p f c", c=4),
                rb[:].rearrange("p (f c) -> p f c", c=4), glfb_b4, op=ALU.mult)
            a_b = mids.tile([P, 4 * FC], bf16)
            nc.scalar.activation(a_b[:], rmb[:], AF.Abs,
                                 accum_out=acc_ab[:, j:j + 1])
            t_b = mids.tile([P, 4 * FC], bf16)
            nc.scalar.activation(t_b[:], a_b[:], AF.Relu, bias=1.0, scale=-1.0)
            q_b = mids.tile([P, 4 * FC], bf16)
            nc.scalar.activation(q_b[:], t_b[:], AF.Square, scale=0.70710678,
                                 accum_out=acc_qb[:, j:j + 1])

            # ---- keypoints ----
            cx_j = bass.AP(tensor=cx_t.tensor, offset=cx_t.offset + j * 2 * FC,
                           ap=[cx_t.ap[0], [2, FC], [0, 5], [1, 2]])
            wk_j = bass.AP(tensor=wk_t.tensor, offset=wk_t.offset + j * 2 * FC,
                           ap=[wk_t.ap[0], [2, FC], [0, 5], [1, 2]])
            gc = mids.tile([P, K * FC], bf16)
            gk_v = gk_t[:].rearrange("p (f a b) -> p f a b", a=5, b=2)
            nc.vector.tensor_tensor(
                gc[:].rearrange("p (f a b) -> p f a b", a=5, b=2),
                gk_v, cx_j, op=ALU.subtract)
            d = mids.tile([P, K * FC], bf16)
            nc.vector.tensor_tensor(
                d[:].rearrange("p (f a b) -> p f a b", a=5, b=2),
                gc[:].rearrange("p (f a b) -> p f a b", a=5, b=2),
                wk_j, op=ALU.mult)
            rk = mids.tile([P, K * FC], bf16)
            nc.vector.tensor_tensor(rk[:], pk_t[:], d[:], op=ALU.subtract)
            # kp mask: glf * (min10 > 0)
            min10 = mids.tile([P, FC], bf16)
            nc.vector.tensor_reduce(min10[:],
                                    gk_t[:].rearrange("p (f c) -> p f c", c=K),
                                    axis=mybir.AxisListType.X, op=ALU.min)
            stp = mids.tile([P, FC], bf16)
            nc.vector.tensor_scalar(stp[:], min10[:], 0.0, None, op0=ALU.is_gt)
            kpm = mids.tile([P, FC], bf16)
            nc.vector.tensor_tensor(kpm[:], stp[:], glfb[:], op=ALU.mult)
            rmk = mids.tile([P, K * FC], bf16)
            kpm_b = bass.AP(tensor=kpm.tensor, offset=kpm.offset,
                            ap=[kpm.ap[0], [1, FC], [0, K]])
            nc.vector.tensor_tensor(
                rmk[:].rearrange("p (f c) -> p f c", c=K),
                rk[:].rearrange("p (f c) -> p f c", c=K), kpm_b, op=ALU.mult)
            a_k = mids.tile([P, K * FC], bf16)
            nc.scalar.activation(a_k[:], rmk[:], AF.Abs,
                                 accum_out=acc_ak[:, j:j + 1])
            t_k = mids.tile([P, K * FC], bf16)
            nc.scalar.activation(t_k[:], a_k[:], AF.Relu, bias=1.0, scale=-1.0)
            q_k = mids.tile([P, K * FC], bf16)
            nc.scalar.activation(q_k[:], t_k[:], AF.Square, scale=0.70710678,
                                 accum_out=acc_qk[:, j:j + 1])

        # ---- finalize: collapse partitions per row ----
        accT = singles.tile([P, 4 * NCH], f32)
        nc.vector.tensor_copy(accT[:, 0:NCH], acc_ab[:])
        nc.vector.tensor_copy(accT[:, NCH:2 * NCH], acc_qb[:])
        nc.vector.tensor_copy(accT[:, 2 * NCH:3 * NCH], acc_ak[:])
        nc.vector.tensor_copy(accT[:, 3 * NCH:4 * NCH], acc_qk[:])
        ps_acc = psum.tile([RPC, 4 * NCH], f32)
        nc.tensor.matmul(ps_acc[:], blk8_t[:], accT[:], start=True, stop=True)

        out_s = singles.tile([RPC, 32], f32)
        nc.vector.memset(out_s[:], 0.0)
        nc.vector.tensor_copy(out_s[:, 0:4 * NCH], ps_acc[:])
        nc.vector.tensor_reduce(out_s[:, 28:29], ps_loss[:],
                                axis=mybir.AxisListType.X, op=ALU.add)
        nc.vector.tensor_reduce(out_s[:, 29:30], ps_pos[:],
                                axis=mybir.AxisListType.X, op=ALU.add)
        nc.vector.tensor_reduce(out_s[:, 30:31], ps_g[:],
                                axis=mybir.AxisListType.X, op=ALU.add)
        nc.sync.dma_start(out[:], out_s[:])

    nc.compile()
    return nc


def _get_nc():
    if "nc" not in _CACHE:
        _CACHE["nc"] = _build()
    return _CACHE["nc"]


def _host_consts(anc):
    anc = np.asarray(anc, np.float32)
    anc_xy = anc[:, :2].reshape(BLK, LA * 2)
    anc_wh = anc[:, 2:]
    cx = anc_xy.astype(ml_dtypes.bfloat16)
    wk = (10.0 / anc_wh).reshape(BLK, LA * 2).astype(ml_dtypes.bfloat16)
    l5c = (5.0 * np.log(anc_wh)).reshape(BLK, LA * 2).astype(np.float32)
    blk8 = np.zeros((P, RPC), np.float32)
    for p in range(P):
        blk8[p, p // BLK] = 1.0
    return cx, wk, l5c, blk8


def _labels_fallback(p_labels, g_labels, rows):
    """Exact numpy recomputation of the OHEM label loss for given rows."""
    res = {}
    for r in rows:
        pl_ = p_labels[r].astype(np.float32)
        gl_ = g_labels[r].astype(np.int64)
        mx = pl_.max(axis=1, keepdims=True)
        lse = mx[:, 0] + np.log(np.exp(pl_ - mx).sum(axis=1))
        loss = lse - pl_[np.arange(A), gl_]
        mask_pos = gl_ > 0
        neg = np.where(mask_pos, 0.0, loss)
        order = np.argsort(-neg, kind="stable")
        rank = np.empty(A, np.int64)
        rank[order] = np.arange(A)
        neg_num = min(3 * int(mask_pos.sum()), A)
        mask_z = mask_pos.astype(np.float32) + (rank < neg_num).astype(np.float32)
        res[r] = float((loss * mask_z).sum())
    return res


def kernel(p_bboxs_xywh, g_bboxs_ltrb, p_labels, g_labels, p_keypoints,
           g_keypoints, anc):
    from concourse.bass_utils import run_bass_kernel_spmd

    nc = _get_nc()
    cx, wk, l5c, blk8 = _host_consts(anc)

    pb = np.ascontiguousarray(np.asarray(p_bboxs_xywh, np.float32))
    gb = np.ascontiguousarray(np.asarray(g_bboxs_ltrb, np.float32))
    pl = np.ascontiguousarray(np.asarray(p_labels, np.float32).astype(ml_dtypes.bfloat16))
    gl = np.ascontiguousarray(np.asarray(g_labels, np.int32))
    pk = np.ascontiguousarray(np.asarray(p_keypoints, np.float32).astype(ml_dtypes.bfloat16))
    gk = np.ascontiguousarray(np.asarray(g_keypoints, np.float32).astype(ml_dtypes.bfloat16))

    in_maps = []
    for c in range(NCORES):
        sl = slice(c * RPC, (c + 1) * RPC)
        in_maps.append({
            "pb": pb[sl], "gb": gb[sl], "pl": pl[sl], "gl": gl[sl],
            "pk": pk[sl], "gk": gk[sl],
            "cx": cx, "wk": wk, "l5c": l5c, "blk8": blk8,
        })

    res = run_bass_kernel_spmd(nc, in_maps, core_ids=list(range(NCORES)))
    _CACHE["last_exec_ns"] = res.exec_time_ns

    O = np.concatenate([res.results[c]["out"] for c in range(NCORES)], axis=0)
    O = O.astype(np.float64)  # host epilogue accumulations
    asum_b = O[:, 0:NCH].sum(1)
    qsum_b = O[:, NCH:2 * NCH].sum(1)
    asum_k = O[:, 2 * NCH:3 * NCH].sum(1)
    qsum_k = O[:, 3 * NCH:4 * NCH].sum(1)
    loss_sum = O[:, 28]
    pos_sum = O[:, 29]
    pos_num = O[:, 30]

    bbox_row = (asum_b + qsum_b) - 2.0 * A
    kp_row = (asum_k + qsum_k) - 5.0 * A
    labels_row = loss_sum + pos_sum

    # OHEM guard: the device shortcut assumed neg_num == A for every row.
    bad = np.where(3.0 * pos_num < A)[0]
    if len(bad):
        fb = _labels_fallback(np.asarray(p_labels, np.float32),
                              np.asarray(g_labels, np.int64), bad)
        for r, v in fb.items():
            labels_row[r] = v

    pos_num = pos_num.astype(np.float32)
    num_mask = (pos_num > 0).astype(np.float32)
    pos_den = np.clip(pos_num, FLOAT_EPS, None)
    scale = num_mask / pos_den
    loss_bboxs = np.float32((bbox_row.astype(np.float32) * scale).mean())
    loss_labels = np.float32((labels_row.astype(np.float32) * scale).mean())
    loss_keypoints = np.float32((kp_row.astype(np.float32) * scale).mean())
    loss_total = np.float32(loss_bboxs + loss_labels + loss_keypoints)
    return loss_total, loss_bboxs, loss_labels, loss_keypoints
